# revision 1
# baseline (speedup 1.0000x reference)
"""Trainium2 Bass kernel for nn_ConformerBlock (B=4, S=4096, D=512).

Sharding: 8 shards = (batch 4) x (sequence halves 2). Each core gets a
2304-token slice (2048 output + 256 halo covering the attention (+-128)
and depthwise-conv (+-15) receptive field) and runs an identical SPMD
program; the host slices each core's valid 2048 tokens and reassembles.
No collectives.

Per-core kernel layout strategy:
  - residual stream: token-major fp32 SBUF tiles [128 tokens, 512]
  - per stage: LayerNorm (DVE bn_stats + ACT per-partition affine, LN
    gamma/beta folded into the next GEMM's weights host-side) -> bf16
    x_hat -> DMA-transpose to D-major [4][128, T] -> weight-stationary
    bf16 GEMMs with biases applied via K=1 ones-row matmuls into PSUM ->
    per-channel nonlinearity on ACT -> last GEMM back to token-major
    PSUM -> fp32 residual add on DVE.
  - attention: scores computed transposed per key-block ([keys, <=384
    queries], N>=256 keeps the PE streaming); rel-position bias and the
    |rel|<=128 window mask folded into host-precomputed B tiles (-1e30
    when masked) added into PSUM via an identity matmul; exp on ACT with
    no max-subtraction (scores are provably small); AV matmuls use a
    ones-augmented V so each head's softmax denominator lands in PSUM
    col 64; per-head normalize on evacuation; out-projection after a
    DMA-transpose.
  - depthwise conv: 31 shifted diagonal-matmul taps accumulated in
    PSUM; BN + SiLU folded into the ACT evacuation (per-channel
    scale/bias on partitions).
"""
import sys
sys.path.insert(0, '/opt/trn_rl_repo')
from contextlib import ExitStack

import numpy as np
import ml_dtypes

import concourse.bass as bass
import concourse.tile as tile
from concourse import bacc, mybir

AF = mybir.ActivationFunctionType
ALU = mybir.AluOpType
FP32 = mybir.dt.float32
BF16 = mybir.dt.bfloat16
EPS = 1e-5

B, S = 4, 4096
D, H, CTX, FFN, KS = 512, 8, 128, 2048, 31
HD = D // H
PAD = 16
NBIAS = 8704
N_TT = 18          # 2304 tokens per shard
HALO = 256         # halo tokens on the interior side
N_CORES = 8

BOFF = {"ff1a": 0, "ff1b": 2048, "ff2a": 2560, "ff2b": 4608,
        "q": 5120, "k": 5632, "v": 6144, "o": 6656, "pw1": 7168,
        "pw2": 8192}


def build_core_kernel(n_tt=N_TT, alo=0, ahi=N_TT, olo=0, ohi=N_TT,
                      act_dt=BF16, reps=1):
    """One core's kernel: n_tt residual tiles of 128 tokens; attention/conv
    over query blocks [alo, ahi); outputs tiles [olo, ohi)."""
    clo, chi = alo, ahi
    T1 = n_tt * 128
    nc = bacc.Bacc("TRN2", target_bir_lowering=False, debug=False, num_devices=1)

    x_ext = nc.dram_tensor("x", [T1, D], FP32, kind="ExternalInput").ap()
    w_ff1a = nc.dram_tensor("w_ff1a", [128, 4 * FFN], act_dt, kind="ExternalInput").ap()
    w_ff1b = nc.dram_tensor("w_ff1b", [128, 16 * D], act_dt, kind="ExternalInput").ap()
    w_ff2a = nc.dram_tensor("w_ff2a", [128, 4 * FFN], act_dt, kind="ExternalInput").ap()
    w_ff2b = nc.dram_tensor("w_ff2b", [128, 16 * D], act_dt, kind="ExternalInput").ap()
    w_qkvo = nc.dram_tensor("w_qkvo", [128, 16 * D], act_dt, kind="ExternalInput").ap()
    w_pw1 = nc.dram_tensor("w_pw1", [128, 4 * 2 * D], act_dt, kind="ExternalInput").ap()
    w_pw2 = nc.dram_tensor("w_pw2", [128, 4 * D], act_dt, kind="ExternalInput").ap()
    w_dw = nc.dram_tensor("w_dw", [128, KS * 4 * 128], act_dt, kind="ExternalInput").ap()
    biasrow_ext = nc.dram_tensor("biasrow", [1, NBIAS], act_dt, kind="ExternalInput").ap()
    bconv_ext = nc.dram_tensor("bconv", [128, 4], FP32, kind="ExternalInput").ap()
    btiles_ext = nc.dram_tensor("btiles", [128, H * 3 * 128], act_dt, kind="ExternalInput").ap()
    ident_ext = nc.dram_tensor("ident", [128, 128], act_dt, kind="ExternalInput").ap()
    y_ext = nc.dram_tensor("y", [(ohi - olo) * 128, D], FP32, kind="ExternalOutput").ap()

    qs = 0.125  # 1/sqrt(HD)

    with tile.TileContext(nc) as tc, ExitStack() as es:
        pool = lambda name, bufs=1, space="SBUF": es.enter_context(
            tc.tile_pool(name=name, bufs=bufs, space=space))

        const_p = pool("const")
        resid_p = pool("resid")
        stat_p = pool("stat", bufs=4)
        xhat_p = pool("xhat", bufs=3)
        gps = pool("gps", bufs=3, space="PSUM")
        sps = pool("sps", bufs=2, space="PSUM")
        aps = pool("aps", bufs=2, space="PSUM")

        ident = const_p.tile([128, 128], act_dt, name="ident")
        nc.gpsimd.dma_start(ident[:], ident_ext[:])
        biasrow = const_p.tile([1, NBIAS], act_dt, name="biasrow")
        nc.gpsimd.dma_start(biasrow[:], biasrow_ext[:])
        onesrow = const_p.tile([1, T1], act_dt, name="onesrow")
        nc.vector.memset(onesrow[:], 1.0)
        bconv = const_p.tile([128, 4], FP32, name="bconv")
        nc.gpsimd.dma_start(bconv[:], bconv_ext[:])
        eps_col = const_p.tile([128, 1], FP32, name="eps_col")
        nc.vector.memset(eps_col[:], EPS)

        def body(rep):
            sfx = f"r{rep}" if reps > 1 else ""
            x_tm = []
            for t in range(n_tt):
                xt = resid_p.tile([128, D], FP32, name=f"x_tm{t}{sfx}", tag=f"x_tm{t}")
                nc.gpsimd.dma_start(xt[:], x_ext[t * 128:(t + 1) * 128, :])
                x_tm.append(xt)

            def ln_stats(t, tag):
                st6 = stat_p.tile([128, 6], FP32, name=f"st6_{tag}{t}{sfx}", tag="st6")
                nc.vector.bn_stats(st6[:], x_tm[t][:])
                st2 = stat_p.tile([128, 2], FP32, name=f"st2_{tag}{t}{sfx}", tag="st2")
                nc.vector.bn_aggr(st2[:], st6[:])
                sig = stat_p.tile([128, 2], FP32, name=f"sig_{tag}{t}{sfx}", tag="sig")
                nc.scalar.activation(sig[:, 0:1], st2[:, 1:2], AF.Sqrt, bias=eps_col[:])
                nc.vector.reciprocal(sig[:, 1:2], sig[:, 0:1])
                nmu = stat_p.tile([128, 1], FP32, name=f"nmu_{tag}{t}{sfx}", tag="nmu")
                nc.vector.tensor_scalar(out=nmu[:], in0=st2[:, 0:1],
                                        scalar1=sig[:, 1:2], scalar2=-1.0,
                                        op0=ALU.mult, op1=ALU.mult)
                return sig, nmu

            def ln_xhatT(tt_lo, tt_hi, wpool, tag):
                width = (tt_hi - tt_lo) * 128
                big = wpool.tile([128, 4 * width], act_dt, name=f"{tag}T{sfx}",
                                 tag=f"{tag}T")
                xT = [big[:, c * width:(c + 1) * width] for c in range(4)]
                big3 = big[:].rearrange("p (c n) -> p c n", c=4)
                for t in range(tt_lo, tt_hi):
                    sig, nmu = ln_stats(t, tag)
                    xh = xhat_p.tile([128, D], act_dt, name=f"xh_{tag}{t}{sfx}", tag="xh")
                    nc.scalar.activation(xh[:], x_tm[t][:], AF.Identity,
                                         bias=nmu[:], scale=sig[:, 1:2])
                    col = (t - tt_lo) * 128
                    nc.sync.dma_start_transpose(big3[:, :, col:col + 128], xh[:])
                return xT

            def nsplit(width):
                out, o = [], 0
                while o < width:
                    w = min(512, width - o)
                    out.append((o, w))
                    o += w
                return out

            def gemm_B(xT, wtile, wcol, m, n_off, n_w, bias_off, nm):
                ps = gps.tile([128, 512], FP32, name=f"psB_{nm}{sfx}", tag="gps")
                nc.tensor.matmul(ps[:, :n_w],
                                 biasrow[:, bias_off + m * 128:bias_off + (m + 1) * 128],
                                 onesrow[:, n_off:n_off + n_w], start=True, stop=False)
                for c in range(4):
                    nc.tensor.matmul(ps[:, :n_w],
                                     wtile[:, c * wcol + m * 128:c * wcol + (m + 1) * 128],
                                     xT[c][:, n_off:n_off + n_w],
                                     start=False, stop=(c == 3))
                return ps

            def gemm_A_tt(parts, rhs_of_c, bias_off, nm):
                ps = gps.tile([128, 512], FP32, name=f"psA_{nm}{sfx}", tag="gps")
                nc.tensor.matmul(ps[:], onesrow[:, 0:128],
                                 biasrow[:, bias_off:bias_off + D], start=True, stop=False)
                for c in range(4):
                    nc.tensor.matmul(ps[:], parts[c], rhs_of_c(c),
                                     start=False, stop=(c == 3))
                return ps

            def ffn_stage(tt_lo, tt_hi, wa_ext, wb_ext, boffa, boffb, tag):
                with tc.tile_pool(name=f"{tag}_sp{sfx}", bufs=1) as sp, \
                     tc.tile_pool(name=f"{tag}_hp{sfx}", bufs=2) as hp:
                    wa = sp.tile([128, 4 * FFN], act_dt, name=f"{tag}_wa{sfx}", tag="wa")
                    nc.gpsimd.dma_start(wa[:], wa_ext[:])
                    wb = sp.tile([128, 16 * D], act_dt, name=f"{tag}_wb{sfx}", tag="wb")
                    nc.gpsimd.dma_start(wb[:], wb_ext[:])
                    xT = ln_xhatT(tt_lo, tt_hi, sp, tag)
                    width = (tt_hi - tt_lo) * 128
                    for (n_off, n_w) in nsplit(width):
                        hs = []
                        for m in range(16):
                            ps = gemm_B(xT, wa, FFN, m, n_off, n_w, boffa,
                                        f"{tag}{m}_{n_off}")
                            h = hp.tile([128, 512], act_dt,
                                        name=f"{tag}_h{m}_{n_off}{sfx}", tag=f"h{m}")
                            nc.scalar.activation(h[:, :n_w], ps[:, :n_w], AF.Gelu)
                            hs.append(h)
                        for sub in range(n_w // 128):
                            tt = tt_lo + (n_off + sub * 128) // 128
                            ps2 = gps.tile([128, 512], FP32,
                                           name=f"{tag}_ps2_{tt}{sfx}", tag="gps")
                            nc.tensor.matmul(ps2[:], onesrow[:, 0:128],
                                             biasrow[:, boffb:boffb + D],
                                             start=True, stop=False)
                            for k in range(16):
                                nc.tensor.matmul(ps2[:],
                                                 hs[k][:, sub * 128:(sub + 1) * 128],
                                                 wb[:, k * D:(k + 1) * D],
                                                 start=False, stop=(k == 15))
                            nc.vector.tensor_add(x_tm[tt][:], x_tm[tt][:], ps2[:])

            def attn_stage():
                with tc.tile_pool(name=f"attn_sp{sfx}", bufs=1) as ap_, \
                     tc.tile_pool(name=f"attn_sp2{sfx}", bufs=2) as ap2:
                    wqkvo = ap_.tile([128, 16 * D], act_dt, name=f"wqkvo{sfx}", tag="wqkvo")
                    nc.gpsimd.dma_start(wqkvo[:], w_qkvo[:])
                    btiles = ap_.tile([128, H * 3 * 128], act_dt,
                                      name=f"btiles{sfx}", tag="btiles")
                    nc.gpsimd.dma_start(btiles[:], btiles_ext[:])
                    xT = ln_xhatT(0, n_tt, ap_, "attn")

                    qT, kT = [], []
                    for nm, woff, dst in (("q", 0, qT), ("k", 4, kT)):
                        boff = BOFF[nm]
                        for m in range(4):
                            dst.append(ap_.tile([128, T1], act_dt,
                                                name=f"{nm}T{m}{sfx}", tag=f"{nm}T{m}"))
                        for (n_off, n_w) in nsplit(T1):
                            for m in range(4):
                                ps = gps.tile([128, 512], FP32,
                                              name=f"ps_{nm}{m}_{n_off}{sfx}", tag="gps")
                                nc.tensor.matmul(
                                    ps[:, :n_w],
                                    biasrow[:, boff + m * 128:boff + (m + 1) * 128],
                                    onesrow[:, n_off:n_off + n_w], start=True, stop=False)
                                for c in range(4):
                                    nc.tensor.matmul(
                                        ps[:, :n_w],
                                        wqkvo[:, (woff + c) * D + m * 128:
                                              (woff + c) * D + (m + 1) * 128],
                                        xT[c][:, n_off:n_off + n_w],
                                        start=False, stop=(c == 3))
                                nc.scalar.activation(dst[m][:, n_off:n_off + n_w],
                                                     ps[:, :n_w], AF.Identity)

                    v_aug = []
                    for t in range(n_tt):
                        va = ap_.tile([128, H * 65], act_dt,
                                      name=f"vaug{t}{sfx}", tag=f"vaug{t}")
                        ps = gemm_A_tt([xT[c][:, t * 128:(t + 1) * 128] for c in range(4)],
                                       lambda c: wqkvo[:, (8 + c) * D:(9 + c) * D],
                                       BOFF["v"], f"v{t}")
                        nc.scalar.activation(
                            va[:].rearrange("p (h w) -> p h w", w=65)[:, :, 0:64],
                            ps[:].rearrange("p (h w) -> p h w", w=64), AF.Identity)
                        nc.vector.memset(
                            va[:].rearrange("p (h w) -> p h w", w=65)[:, :, 64:65], 1.0)
                        v_aug.append(va)

                    expw = {}
                    awidth = (ahi - alo) * 128
                    attnT_big = ap_.tile([128, 4 * awidth], act_dt,
                                         name=f"attnT{sfx}", tag="attnT")
                    attnT = [attnT_big[:, c * awidth:(c + 1) * awidth] for c in range(4)]
                    attnT3 = attnT_big[:].rearrange("p (c n) -> p c n", c=4)

                    def do_av(qb):
                        kbs = [kb for kb in (qb - 1, qb, qb + 1) if 0 <= kb < n_tt]
                        atm = ap2.tile([128, D], act_dt, name=f"atm{qb}{sfx}", tag="atm")
                        for hgrp in range(2):
                            pa = aps.tile([128, 4 * 65], FP32,
                                          name=f"pav{qb}_{hgrp}{sfx}", tag="aps")
                            for hh in range(4):
                                h = hgrp * 4 + hh
                                for i, kb in enumerate(kbs):
                                    ew, lo_qb = expw[(kb, h)]
                                    nc.tensor.matmul(
                                        pa[:, hh * 65:(hh + 1) * 65],
                                        ew[:, (qb - lo_qb) * 128:(qb - lo_qb + 1) * 128],
                                        v_aug[kb][:, h * 65:(h + 1) * 65],
                                        start=(i == 0), stop=(i == len(kbs) - 1),
                                        skip_group_check=True)
                            for hh in range(4):
                                h = hgrp * 4 + hh
                                rc = stat_p.tile([128, 1], FP32,
                                                 name=f"rc{qb}_{h}{sfx}", tag="rc")
                                nc.vector.reciprocal(rc[:], pa[:, hh * 65 + 64:hh * 65 + 65])
                                if h % 2 == 0:
                                    nc.vector.tensor_scalar(
                                        out=atm[:, h * 64:(h + 1) * 64],
                                        in0=pa[:, hh * 65:hh * 65 + 64],
                                        scalar1=rc[:], scalar2=None, op0=ALU.mult)
                                else:
                                    nc.scalar.activation(
                                        atm[:, h * 64:(h + 1) * 64],
                                        pa[:, hh * 65:hh * 65 + 64], AF.Identity,
                                        scale=rc[:])
                        col = (qb - alo) * 128
                        nc.sync.dma_start_transpose(attnT3[:, :, col:col + 128], atm[:])

                    for kb in range(n_tt + 1):
                        if kb < n_tt:
                            lo_qb = max(kb - 1, alo)
                            hi_qb = min(kb + 1, ahi - 1)
                            if lo_qb <= hi_qb:
                                ncols = (hi_qb - lo_qb + 1) * 128
                                for h in range(H):
                                    pss = sps.tile([128, 384], FP32,
                                                   name=f"pss{kb}_{h}{sfx}", tag="sps")
                                    boff2 = (lo_qb - (kb - 1)) * 128
                                    nc.tensor.matmul(
                                        pss[:, :ncols], ident[:],
                                        btiles[:, h * 384 + boff2:h * 384 + boff2 + ncols],
                                        start=True, stop=False)
                                    hrow = (h % 2) * 64
                                    nc.tensor.matmul(
                                        pss[:, :ncols],
                                        kT[h // 2][hrow:hrow + 64, kb * 128:(kb + 1) * 128],
                                        qT[h // 2][hrow:hrow + 64,
                                                   lo_qb * 128:lo_qb * 128 + ncols],
                                        start=False, stop=True)
                                    ew = ap2.tile([128, 384], act_dt,
                                                  name=f"ew{kb}_{h}{sfx}",
                                                  tag=f"ew{h}", bufs=3)
                                    nc.scalar.activation(ew[:, :ncols], pss[:, :ncols],
                                                         AF.Exp, scale=qs)
                                    expw[(kb, h)] = (ew, lo_qb)
                        qb = kb - 1
                        if alo <= qb < ahi:
                            do_av(qb)

                    for tt in range(alo, ahi):
                        ps2 = gemm_A_tt(
                            [attnT[c][:, (tt - alo) * 128:(tt - alo + 1) * 128]
                             for c in range(4)],
                            lambda c: wqkvo[:, (12 + c) * D:(13 + c) * D],
                            BOFF["o"], f"wo{tt}")
                        nc.vector.tensor_add(x_tm[tt][:], x_tm[tt][:], ps2[:])

            def conv_stage():
                with tc.tile_pool(name=f"conv_sp{sfx}", bufs=1) as cp, \
                     tc.tile_pool(name=f"conv_sp2{sfx}", bufs=2) as cp2:
                    wpw1 = cp.tile([128, 4 * 2 * D], act_dt, name=f"wpw1{sfx}", tag="wpw1")
                    nc.gpsimd.dma_start(wpw1[:], w_pw1[:])
                    wpw2 = cp.tile([128, 4 * D], act_dt, name=f"wpw2{sfx}", tag="wpw2")
                    nc.gpsimd.dma_start(wpw2[:], w_pw2[:])
                    wdw = cp.tile([128, KS * 4 * 128], act_dt, name=f"wdw{sfx}", tag="wdw")
                    nc.gpsimd.dma_start(wdw[:], w_dw[:])
                    xT = ln_xhatT(clo, chi, cp, "conv")
                    Tc = (chi - clo) * 128
                    hg = [cp.tile([128, Tc + 2 * PAD], act_dt,
                                  name=f"hg{c}{sfx}", tag=f"hg{c}") for c in range(4)]
                    for c in range(4):
                        nc.vector.memset(hg[c][:, 0:PAD], 0.0)
                        nc.vector.memset(hg[c][:, PAD + Tc:], 0.0)
                    for (n_off, n_w) in nsplit(Tc):
                        gates = []
                        for m in range(4):
                            psg = gemm_B(xT, wpw1, 2 * D, 4 + m, n_off, n_w,
                                         BOFF["pw1"], f"g{m}_{n_off}")
                            g = cp2.tile([128, 512], act_dt,
                                         name=f"gate{m}_{n_off}{sfx}", tag=f"gate{m}")
                            nc.scalar.activation(g[:, :n_w], psg[:, :n_w], AF.Sigmoid)
                            gates.append(g)
                        for m in range(4):
                            psa = gemm_B(xT, wpw1, 2 * D, m, n_off, n_w,
                                         BOFF["pw1"], f"a{m}_{n_off}")
                            nc.vector.tensor_mul(hg[m][:, PAD + n_off:PAD + n_off + n_w],
                                                 psa[:, :n_w], gates[m][:, :n_w])
                    for (n_off, n_w) in nsplit(Tc):
                        sl = []
                        for c in range(4):
                            psd = gps.tile([128, 512], FP32,
                                           name=f"psd{c}_{n_off}{sfx}", tag="gps")
                            for k in range(KS):
                                nc.tensor.matmul(
                                    psd[:, :n_w],
                                    wdw[:, (k * 4 + c) * 128:(k * 4 + c + 1) * 128],
                                    hg[c][:, PAD + n_off + k - (KS // 2):
                                          PAD + n_off + k - (KS // 2) + n_w],
                                    start=(k == 0), stop=(k == KS - 1))
                            s = cp2.tile([128, 512], act_dt,
                                         name=f"sl{c}_{n_off}{sfx}", tag=f"sl{c}")
                            nc.scalar.activation(s[:, :n_w], psd[:, :n_w], AF.Silu,
                                                 bias=bconv[:, c:c + 1])
                            sl.append(s)
                        for sub in range(n_w // 128):
                            tt = clo + (n_off + sub * 128) // 128
                            ps2 = gemm_A_tt(
                                [sl[c][:, sub * 128:(sub + 1) * 128] for c in range(4)],
                                lambda c: wpw2[:, c * D:(c + 1) * D],
                                BOFF["pw2"], f"pw2_{tt}")
                            nc.vector.tensor_add(x_tm[tt][:], x_tm[tt][:], ps2[:])

            def final_stage():
                for t in range(olo, ohi):
                    sig, nmu = ln_stats(t, "fin")
                    yt = xhat_p.tile([128, D], FP32, name=f"yt{t}{sfx}", tag="yt")
                    nc.scalar.activation(yt[:], x_tm[t][:], AF.Identity,
                                         bias=nmu[:], scale=sig[:, 1:2])
                    nc.gpsimd.dma_start(y_ext[(t - olo) * 128:(t - olo + 1) * 128, :], yt[:])

            ffn_stage(0, n_tt, w_ff1a, w_ff1b, BOFF["ff1a"], BOFF["ff1b"], "ff1")
            attn_stage()
            conv_stage()
            ffn_stage(olo, ohi, w_ff2a, w_ff2b, BOFF["ff2a"], BOFF["ff2b"], "ff2")
            final_stage()

        for rep in range(reps):
            body(rep)

    nc.compile()
    return nc


# ===================== host-side preprocessing =====================

def _pack_rows(w):
    din, dout = w.shape
    return np.ascontiguousarray(
        w.reshape(din // 128, 128, dout).transpose(1, 0, 2).reshape(128, -1))


def prep_weights(inp, act_np=ml_dtypes.bfloat16):
    f = lambda a: np.asarray(a, dtype=np.float32)
    out = {}
    biasrow = np.zeros(NBIAS, np.float32)

    def fold_ln(g, b, w, bias):
        return f(g)[:, None] * f(w), f(b) @ f(w) + f(bias)

    for p, wa_k, wb_k, boffa, boffb in (
            ("ff1", "w_ff1a", "w_ff1b", BOFF["ff1a"], BOFF["ff1b"]),
            ("ff2", "w_ff2a", "w_ff2b", BOFF["ff2a"], BOFF["ff2b"])):
        w1g, b1 = fold_ln(inp[p + "_ln_g"], inp[p + "_ln_b"], inp[p + "_w1"], inp[p + "_b1"])
        out[wa_k] = _pack_rows(w1g).astype(act_np)
        biasrow[boffa:boffa + FFN] = b1
        out[wb_k] = _pack_rows(f(inp[p + "_w2"]) * 0.5).astype(act_np)
        biasrow[boffb:boffb + D] = f(inp[p + "_b2"]) * 0.5

    g, b = inp["attn_ln_g"], inp["attn_ln_b"]
    packs = []
    for nm in ("q", "k", "v"):
        wg, bb = fold_ln(g, b, inp["w" + nm], inp["b" + nm])
        packs.append(_pack_rows(wg))
        biasrow[BOFF[nm]:BOFF[nm] + D] = bb
    packs.append(_pack_rows(f(inp["wo"])))
    biasrow[BOFF["o"]:BOFF["o"] + D] = f(inp["bo"])
    out["w_qkvo"] = np.concatenate(packs, axis=1).astype(act_np)

    wg, bb = fold_ln(inp["conv_ln_g"], inp["conv_ln_b"], inp["pw1_w"], inp["pw1_b"])
    out["w_pw1"] = _pack_rows(wg).astype(act_np)
    biasrow[BOFF["pw1"]:BOFF["pw1"] + 2 * D] = bb
    out["w_pw2"] = _pack_rows(f(inp["pw2_w"])).astype(act_np)
    biasrow[BOFF["pw2"]:BOFF["pw2"] + D] = f(inp["pw2_b"])

    bn_scale = f(inp["bn_g"]) / np.sqrt(f(inp["bn_v"]) + EPS)
    dww = f(inp["dw_w"])[:, 0, :] * bn_scale[:, None]
    bconv_full = (f(inp["dw_b"]) - f(inp["bn_m"])) * bn_scale + f(inp["bn_b"])
    wdw = np.zeros((128, KS * 4 * 128), np.float32)
    for k in range(KS):
        for c in range(4):
            blk = wdw[:, (k * 4 + c) * 128:(k * 4 + c + 1) * 128]
            np.fill_diagonal(blk, dww[c * 128:(c + 1) * 128, k])
    out["w_dw"] = wdw.astype(act_np)
    out["bconv"] = np.ascontiguousarray(bconv_full.reshape(4, 128).T).astype(np.float32)

    rb = f(inp["rel_bias"])
    j = np.arange(128)[:, None]
    i = np.arange(128)[None, :]
    bt = np.zeros((128, H * 3 * 128), np.float32)
    for h in range(H):
        for di, delta in enumerate((1, 0, -1)):
            rel = delta * 128 + j - i
            valid = np.abs(rel) <= CTX
            idx = np.clip(rel + CTX, 0, 2 * CTX)
            bt[:, h * 384 + di * 128:h * 384 + (di + 1) * 128] = \
                np.where(valid, 8.0 * rb[h, idx], -1e30)
    out["btiles"] = bt.astype(act_np)
    out["ident"] = np.eye(128, dtype=np.float32).astype(act_np)
    out["biasrow"] = biasrow[None, :].astype(act_np)
    return out


# ===================== SPMD runner =====================

def _make_runner(nc, n_cores):
    import jax
    from jax.sharding import Mesh, PartitionSpec
    from jax.experimental.shard_map import shard_map
    from concourse import bass2jax
    from concourse.bass2jax import _bass_exec_p, install_neuronx_cc_hook

    install_neuronx_cc_hook()
    partition_name = nc.partition_id_tensor.name if nc.partition_id_tensor else None
    in_names, out_names, out_avals, zero_shapes = [], [], [], []
    for alloc in nc.m.functions[0].allocations:
        if not isinstance(alloc, mybir.MemoryLocationSet):
            continue
        name = alloc.memorylocations[0].name
        if alloc.kind == "ExternalInput":
            if name != partition_name:
                in_names.append(name)
        elif alloc.kind == "ExternalOutput":
            out_names.append(name)
            shape = tuple(alloc.tensor_shape)
            dtype = mybir.dt.np(alloc.dtype)
            out_avals.append(jax.core.ShapedArray(shape, dtype))
            zero_shapes.append((shape, dtype))
    n_params = len(in_names)
    n_outs = len(out_avals)
    all_in_names = list(in_names) + list(out_names)
    if partition_name is not None:
        all_in_names.append(partition_name)

    def _body(*args):
        operands = list(args)
        if partition_name is not None:
            operands.append(bass2jax.partition_id_tensor())
        outs = _bass_exec_p.bind(
            *operands, out_avals=tuple(out_avals), in_names=tuple(all_in_names),
            out_names=tuple(out_names), lowering_input_output_aliases=(),
            sim_require_finite=True, sim_require_nnan=True, nc=nc)
        return tuple(outs)

    devices = jax.devices()[:n_cores]
    mesh = Mesh(np.asarray(devices), ("core",))
    sharded = jax.jit(
        shard_map(_body, mesh=mesh,
                  in_specs=(PartitionSpec("core"),) * (n_params + n_outs),
                  out_specs=(PartitionSpec("core"),) * n_outs, check_rep=False),
        donate_argnums=tuple(range(n_params, n_params + n_outs)),
        keep_unused=True)

    def run(in_maps):
        per_core = [[np.asarray(m[n]) for n in in_names] for m in in_maps]
        concat_in = [np.concatenate([per_core[c][i] for c in range(n_cores)], axis=0)
                     for i in range(n_params)]
        concat_zeros = [np.zeros((n_cores * s[0], *s[1:]), d) for (s, d) in zero_shapes]
        out_arrs = sharded(*concat_in, *concat_zeros)
        out_arrs = [np.asarray(o) for o in out_arrs]
        return [{name: out_arrs[i].reshape(n_cores, *out_avals[i].shape)[c]
                 for i, name in enumerate(out_names)}
                for c in range(n_cores)]

    return run


_CACHE = {}


def _get_compiled(reps=1):
    key = ("main", reps)
    if key not in _CACHE:
        nc = build_core_kernel(reps=reps)
        _CACHE[key] = _make_runner(nc, N_CORES)
    return _CACHE[key]


def kernel(**inputs):
    x = np.asarray(inputs["x"], dtype=np.float32)  # [B, S, D]
    wmap = prep_weights(inputs)
    T1 = N_TT * 128
    in_maps = []
    for b in range(B):
        for half in range(2):
            start = 0 if half == 0 else S - T1
            m = dict(wmap)
            m["x"] = np.ascontiguousarray(x[b, start:start + T1])
            in_maps.append(m)
    run = _get_compiled()
    res = run(in_maps)
    y = np.empty((B, S, D), dtype=np.float32)
    for idx in range(N_CORES):
        b, half = divmod(idx, 2)
        out = res[idx]["y"]  # [T1, D]
        if half == 0:
            y[b, 0:S // 2] = out[0:S // 2]
        else:
            y[b, S // 2:] = out[T1 - S // 2:]
    return y



# revision 13
# speedup vs baseline: 1.8336x; 1.8336x over previous
"""Trainium2 Bass kernel for nn_ConformerBlock (B=4, S=4096, D=512).

Sharding: 8 shards = (batch 4) x (sequence halves 2), each core owns a
2304-token slice (2048 valid + 256 halo). SPMD, no collectives.

v2 design (fp8 DoubleRow):
  - All big GEMMs run fp8e4m3 with MatmulPerfMode.DoubleRow: each
    instruction contracts 2 k-tiles at 0.5 PE-cycles/out-col (4x fewer
    PE cycles than bf16).
  - fp8 denormal avoidance: every weight matrix is stored x32. The
    residual stream x_tm is kept at 32x true scale (LayerNorm is
    scale-invariant so this costs one entry-scale pass); with that,
    second-GEMM PSUM results (32 x branch) add directly onto the 32x
    residual, and first-GEMM evacuations absorb 1/32 in the ACT scale.
  - Attention: q/k evacuated at true scale (DVE, scale 1/32); scores
    get rel-bias+mask via an identity matmul of bf16 btiles; exp on
    ACT; v kept at 32x with the softmax-denominator ones column also
    32.0 so the normalization cancels the scale.
  - Depthwise conv: 31 taps as 16 DoubleRow diagonal-pair matmuls with
    overlapping-stride moving APs (rhs dim1 stride = 1 element shift).
  - Engine balance: Gelu/Exp/Sigmoid/Silu evacs on ACT (table funcs,
    batched per stage to avoid act-table reloads; LN sqrt batched per
    stage); x_hat/LN stats/residual adds/q+k evacs on DVE; fp8
    converts, v scatter and entry-scale on Pool (gpsimd); all DMAs on
    the sync engine.
"""
import sys
sys.path.insert(0, '/opt/trn_rl_repo')
from contextlib import ExitStack

import numpy as np
import ml_dtypes

import concourse.bass as bass
import concourse.tile as tile
from concourse import bacc, mybir
from concourse.bass_types import AP

AF = mybir.ActivationFunctionType
ALU = mybir.AluOpType
PM = mybir.MatmulPerfMode
FP32 = mybir.dt.float32
BF16 = mybir.dt.bfloat16
FP8 = mybir.dt.float8e4
EPS = 1e-5

B, S = 4, 4096
D, H, CTX, FFN, KS = 512, 8, 128, 2048, 31
HD = D // H
PAD = 16
N_TT = 18          # 2304 tokens per shard
N_CORES = 8
T1 = N_TT * 128
WS = 32.0          # weight/residual scale
QS = 0.125         # 1/sqrt(HD)

# bias_cols column map (true-scale per-partition biases)
BC = {"ff1": 0, "ff2": 16, "q": 32, "k": 36, "v": 40, "pw1": 44, "conv": 52}
# biasrow segment map (32x-scale free-dim bias rows, 512 wide each)
BR = {"ff1": 0, "o": 512, "pw2": 1024, "ff2": 1536}


def nsplit(width, step=512):
    out, o = [], 0
    while o < width:
        w = min(step, width - o)
        out.append((o, w))
        o += w
    return out


def build_core_kernel(n_tt=N_TT, reps=1):
    nc = bacc.Bacc("TRN2", target_bir_lowering=False, debug=False, num_devices=1)

    x_ext = nc.dram_tensor("x", [T1, D], FP32, kind="ExternalInput").ap()
    w_ffa_ext = [nc.dram_tensor(f"w_ff{i}a", [128, 8192], FP8, kind="ExternalInput").ap()
                 for i in (1, 2)]
    w_ffb_ext = [nc.dram_tensor(f"w_ff{i}b", [128, 8192], FP8, kind="ExternalInput").ap()
                 for i in (1, 2)]
    w_qkv_ext = nc.dram_tensor("w_qkv", [128, 6144], FP8, kind="ExternalInput").ap()
    w_o_ext = nc.dram_tensor("w_o", [128, 2048], FP8, kind="ExternalInput").ap()
    w_pw1_ext = nc.dram_tensor("w_pw1", [128, 4096], FP8, kind="ExternalInput").ap()
    w_pw2_ext = nc.dram_tensor("w_pw2", [128, 2048], FP8, kind="ExternalInput").ap()
    w_dw_ext = nc.dram_tensor("w_dw", [128, 16384], FP8, kind="ExternalInput").ap()
    btiles_ext = nc.dram_tensor("btiles", [128, H * 3 * 128], BF16, kind="ExternalInput").ap()
    ident_ext = nc.dram_tensor("ident", [128, 128], BF16, kind="ExternalInput").ap()
    biasrow_ext = nc.dram_tensor("biasrow", [1, 2048], BF16, kind="ExternalInput").ap()
    bcols_ext = nc.dram_tensor("bcols", [128, 64], FP32, kind="ExternalInput").ap()
    y_ext = nc.dram_tensor("y", [T1, D], FP32, kind="ExternalOutput").ap()

    def drp_n(t, off, s1, n):
        base = t[:]
        return AP(base.tensor, base.offset + off, [list(base.ap[0]), [s1, 2], [1, n]])

    with tile.TileContext(nc) as tc, ExitStack() as es:
        pool = lambda name, bufs=1, space="SBUF": es.enter_context(
            tc.tile_pool(name=name, bufs=bufs, space=space))

        const_p = pool("const")
        wp = pool("wp")            # weights (wa/wb shared between ff1/ff2)
        resid_p = pool("resid")
        stat_p = pool("stat", bufs=2)
        xt_p = pool("xt")          # xT bf16 + fp8
        xh_p = pool("xh", bufs=3)
        hp = pool("hp", bufs=2)    # h8 chunks, g8 chunks, atm tiles
        attn_p = pool("attn")
        big_ps = pool("big_ps", bufs=2, space="PSUM")    # [128,1024] first gemms
        out_ps = pool("out_ps", bufs=2, space="PSUM")    # [128,512] second gemms/v/av
        sps = pool("sps", bufs=2, space="PSUM")          # [128,384] scores

        ident = const_p.tile([128, 128], BF16, name="ident")
        nc.sync.dma_start(ident[:], ident_ext[:])
        ones1 = const_p.tile([1, 128], BF16, name="ones1")
        nc.vector.memset(ones1[:], 1.0)
        biasrow = const_p.tile([1, 2048], BF16, name="biasrow")
        nc.sync.dma_start(biasrow[:], biasrow_ext[:])
        bcols = const_p.tile([128, 64], FP32, name="bcols")
        nc.sync.dma_start(bcols[:], bcols_ext[:])
        eps_col = const_p.tile([128, 1], FP32, name="eps_col")
        nc.vector.memset(eps_col[:], EPS * WS * WS)

        # persistent weight tiles
        wa = wp.tile([128, 8192], FP8, name="wa", tag="wa")
        wb = wp.tile([128, 8192], FP8, name="wb", tag="wb")
        wqkv = wp.tile([128, 6144], FP8, name="wqkv")
        wo = wp.tile([128, 2048], FP8, name="wo")
        wpw1 = wp.tile([128, 4096], FP8, name="wpw1")
        wpw2 = wp.tile([128, 2048], FP8, name="wpw2")
        wdw = wp.tile([128, 16384], FP8, name="wdw")
        btiles = wp.tile([128, H * 3 * 128], BF16, name="btiles")

        def body(rep):
            sfx = f"r{rep}" if reps > 1 else ""
            xbig = resid_p.tile([128, n_tt * D], FP32, name=f"xbig{sfx}", tag="xbig")
            x3 = xbig[:].rearrange("p (t d) -> p t d", t=n_tt)
            xsrc = x_ext[:].rearrange("(t p) d -> p t d", p=128)

            # x load + entry scale (x32), in blocks of 3 tiles, sync DMA + Pool scale
            for blk in range(6):
                t0, t1_ = blk * 3, blk * 3 + 3
                nc.sync.dma_start(x3[:, t0:t1_, :], xsrc[:, t0:t1_, :])
                nc.gpsimd.tensor_scalar(
                    out=xbig[:, t0 * D:t1_ * D], in0=xbig[:, t0 * D:t1_ * D],
                    scalar1=WS, scalar2=None, op0=ALU.mult)

            # weight loads (sync engine; ff1 first so compute can start)
            nc.sync.dma_start(wa[:], w_ffa_ext[0][:])
            nc.sync.dma_start(wb[:], w_ffb_ext[0][:])
            nc.sync.dma_start(wqkv[:], w_qkv_ext[:])
            nc.sync.dma_start(btiles[:], btiles_ext[:])
            nc.sync.dma_start(wo[:], w_o_ext[:])
            nc.sync.dma_start(wpw1[:], w_pw1_ext[:])
            nc.sync.dma_start(wdw[:], w_dw_ext[:])
            nc.sync.dma_start(wpw2[:], w_pw2_ext[:])

            x8T = xt_p.tile([128, 4 * T1], FP8, name=f"x8T{sfx}", tag="x8T")
            x8T3 = x8T[:].rearrange("p (c n) -> p c n", c=4)

            def ln_xhat(tag, fin=False):
                """LN of xbig -> bf16 xTbf (transposed) -> fp8 x8T.
                Batched stats: bn_stats/aggr per tile -> one sqrt/recip/nmu.
                If fin: writes fp32 y back into xbig instead."""
                st2g = stat_p.tile([128, n_tt * 2], FP32, name=f"st2_{tag}{sfx}", tag="st2g")
                sigg = stat_p.tile([128, 2 * n_tt], FP32, name=f"sig_{tag}{sfx}", tag="sigg")
                nmug = stat_p.tile([128, n_tt], FP32, name=f"nmu_{tag}{sfx}", tag="nmug")
                for t in range(n_tt):
                    st6 = stat_p.tile([128, 6], FP32, name=f"st6_{tag}{t}{sfx}", tag="st6")
                    nc.vector.bn_stats(st6[:], x3[:, t, :])
                    nc.vector.bn_aggr(st2g[:, 2 * t:2 * t + 2], st6[:])
                # sigg[:, :n_tt] = sqrt(var+eps); sigg[:, n_tt:] = 1/sig
                varv = AP(st2g[:].tensor, st2g[:].offset + 1,
                          [list(st2g[:].ap[0]), [2, n_tt]])
                nc.scalar.activation(sigg[:, 0:n_tt], varv, AF.Sqrt, bias=eps_col[:])
                nc.vector.reciprocal(sigg[:, n_tt:2 * n_tt], sigg[:, 0:n_tt])
                meanv = AP(st2g[:].tensor, st2g[:].offset,
                           [list(st2g[:].ap[0]), [2, n_tt]])
                nc.vector.scalar_tensor_tensor(
                    out=nmug[:], in0=meanv, scalar=-1.0,
                    in1=sigg[:, n_tt:2 * n_tt], op0=ALU.mult, op1=ALU.mult)
                for t in range(n_tt):
                    if fin:
                        nc.vector.tensor_scalar(
                            out=x3[:, t, :], in0=x3[:, t, :],
                            scalar1=sigg[:, n_tt + t:n_tt + t + 1],
                            scalar2=nmug[:, t:t + 1], op0=ALU.mult, op1=ALU.add)
                    else:
                        xh = xh_p.tile([128, D], BF16, name=f"xh_{tag}{t}{sfx}", tag="xh")
                        nc.vector.tensor_scalar(
                            out=xh[:], in0=x3[:, t, :],
                            scalar1=sigg[:, n_tt + t:n_tt + t + 1],
                            scalar2=nmug[:, t:t + 1], op0=ALU.mult, op1=ALU.add)
                        xtb = xh_p.tile([128, 4 * 128], BF16,
                                        name=f"xtb_{tag}{t}{sfx}", tag="xtb")
                        xtb3 = xtb[:].rearrange("p (c n) -> p c n", c=4)
                        nc.sync.dma_start_transpose(xtb3[:, :, :], xh[:])
                        nc.gpsimd.tensor_scalar(
                            out=x8T3[:, :, t * 128:(t + 1) * 128], in0=xtb3[:, :, :],
                            scalar1=1.0, scalar2=None, op0=ALU.bypass)

            def first_gemm(wtile, wcoloff, m, n_off, n_w, ps, po=0):
                """chan-major GEMM: out[m-block, tokens n_off:n_off+n_w] into
                ps[:, po:po+n_w]. K=512 via 2 DoubleRow pairs."""
                for p in range(2):
                    lhs = drp_n(wtile, wcoloff + (m * 2 + p) * 256, 128, 128)
                    for (so, sw) in nsplit(n_w, 512):
                        rhs = drp_n(x8T, (2 * p) * T1 + n_off + so, T1, sw)
                        nc.tensor.matmul(ps[:, po + so:po + so + sw], lhs, rhs,
                                         start=(p == 0), stop=(p == 1),
                                         perf_mode=PM.DoubleRow,
                                         skip_group_check=True)

            def second_gemm_ps(lhstile, lhs_off, lhs_s1, npair, wtile, nm):
                """token-major GEMM: ps2[tok,512] = bias + sum DR pairs."""
                ps2 = out_ps.tile([128, 512], FP32, name=f"ps2_{nm}{sfx}", tag="out_ps")
                nc.tensor.matmul(ps2[:], ones1[:], biasrow[0:1, BR[nm.split('_')[0]]:
                                 BR[nm.split('_')[0]] + 512], start=True, stop=False)
                for j in range(npair):
                    lhs = drp_n(lhstile, lhs_off + j * 2 * lhs_s1, lhs_s1, 128)
                    rhs = drp_n(wtile, j * 1024, 512, 512)
                    nc.tensor.matmul(ps2[:], lhs, rhs, start=False, stop=(j == npair - 1),
                                     perf_mode=PM.DoubleRow)
                return ps2

            def ffn_stage(idx, tag):
                if idx == 1:
                    nc.sync.dma_start(wa[:], w_ffa_ext[1][:])
                    nc.sync.dma_start(wb[:], w_ffb_ext[1][:])
                ln_xhat(tag)
                for (n_off, n_w) in nsplit(T1, 1024):
                    h8 = hp.tile([128, 16 * 1024], FP8, name=f"h8_{tag}{n_off}{sfx}", tag="h8")
                    h83 = h8[:].rearrange("p (k n) -> p k n", k=16)
                    for m in range(16):
                        ps = big_ps.tile([128, 1024], FP32,
                                         name=f"ps_{tag}{m}_{n_off}{sfx}", tag="big_ps")
                        first_gemm(wa, 0, m, n_off, n_w, ps)
                        nc.scalar.activation(h83[:, m, 0:n_w], ps[:, 0:n_w], AF.Gelu,
                                             bias=bcols[:, BC[tag] + m:BC[tag] + m + 1],
                                             scale=1.0 / WS)
                    for sub in range(n_w // 128):
                        t = (n_off + sub * 128) // 128
                        ps2 = second_gemm_ps(h8, sub * 128, 1024, 8, wb, f"{tag}_{t}")
                        nc.vector.tensor_tensor(out=x3[:, t, :], in0=ps2[:],
                                                in1=x3[:, t, :], op=ALU.add)

            def attn_stage():
                ln_xhat("attn")
                qkT = []   # q, k fp8 true-scale, [128, T1] x4 m-tiles each
                for pi, nm in ((0, "q"), (1, "k")):
                    tiles = []
                    for m in range(4):
                        qt = attn_p.tile([128, T1], FP8, name=f"{nm}T{m}{sfx}",
                                         tag=f"{nm}T{m}")
                        tiles.append(qt)
                        for (n_off, n_w) in nsplit(T1, 1024):
                            ps = big_ps.tile([128, 1024], FP32,
                                             name=f"ps_{nm}{m}_{n_off}{sfx}", tag="big_ps")
                            first_gemm(wqkv, pi * 2048, m, n_off, n_w, ps)
                            nc.vector.tensor_scalar(
                                out=qt[:, n_off:n_off + n_w], in0=ps[:, 0:n_w],
                                scalar1=1.0 / WS,
                                scalar2=bcols[:, BC[nm] + m:BC[nm] + m + 1],
                                op0=ALU.mult, op1=ALU.add)
                    qkT.append(tiles)
                qT, kT = qkT

                # v (32x) into augmented layout, via token-major gemm per tile
                vab = attn_p.tile([128, n_tt * 8 * 65], FP8, name=f"vab{sfx}", tag="abig2")
                va3 = vab[:].rearrange("p (t h w) -> p t h w", t=n_tt, h=8)
                for t in range(n_tt):
                    ps = out_ps.tile([128, 512], FP32, name=f"psv{t}{sfx}", tag="out_ps")
                    for p in range(2):
                        lhs = drp_n(x8T, (2 * p) * T1 + t * 128, T1, 128)
                        rhs = drp_n(wqkv, 2 * 2048 + p * 1024, 512, 512)
                        nc.tensor.matmul(ps[:], lhs, rhs, start=(p == 0), stop=(p == 1),
                                         perf_mode=PM.DoubleRow)
                    # scatter 8x64 into 8x65 slots (Pool), bias add (32x scale: v=32(v+bv))
                    nc.gpsimd.tensor_scalar(
                        out=va3[:, t, :, 0:64],
                        in0=ps[:].rearrange("p (h w) -> p h w", h=8),
                        scalar1=1.0, scalar2=None, op0=ALU.bypass)
                    nc.vector.memset(va3[:, t, :, 64:65], WS)

                at8 = attn_p.tile([128, 4 * T1], FP8, name=f"at8{sfx}", tag="abig1")
                at83 = at8[:].rearrange("p (c n) -> p c n", c=4)

                expw = {}

                def do_av(qb):
                    atm = xh_p.tile([128, D], BF16, name=f"atm{qb}{sfx}", tag="atm")
                    kbs = [kb for kb in (qb - 1, qb, qb + 1) if 0 <= kb < n_tt]
                    for hgrp in range(2):
                        pa = out_ps.tile([128, 512], FP32,
                                         name=f"pav{qb}_{hgrp}{sfx}", tag="out_ps")
                        for hh in range(4):
                            h = hgrp * 4 + hh
                            for i, kb in enumerate(kbs):
                                ew, lo_qb = expw[(kb, h)]
                                nc.tensor.matmul(
                                    pa[:, hh * 65:hh * 65 + 65],
                                    ew[:, (qb - lo_qb) * 128:(qb - lo_qb + 1) * 128],
                                    va3[:, kb, h, :],
                                    start=(i == 0), stop=(i == len(kbs) - 1),
                                    skip_group_check=True)
                        rc = stat_p.tile([128, 4], FP32, name=f"rc{qb}_{hgrp}{sfx}", tag="rc")
                        dens = AP(pa[:].tensor, pa[:].offset + 64,
                                  [list(pa[:].ap[0]), [65, 4]])
                        nc.vector.reciprocal(rc[:], dens)
                        for hh in range(4):
                            h = hgrp * 4 + hh
                            eng = nc.vector if hh % 2 == 0 else nc.gpsimd
                            eng.tensor_scalar(
                                out=atm[:, h * 64:(h + 1) * 64],
                                in0=pa[:, hh * 65:hh * 65 + 64],
                                scalar1=rc[:, hh:hh + 1], scalar2=None, op0=ALU.mult)
                    atb = xh_p.tile([128, 4 * 128], BF16, name=f"atb{qb}{sfx}", tag="xtb")
                    atb3 = atb[:].rearrange("p (c n) -> p c n", c=4)
                    nc.sync.dma_start_transpose(atb3[:, :, :], atm[:])
                    nc.gpsimd.tensor_scalar(
                        out=at83[:, :, qb * 128:(qb + 1) * 128], in0=atb3[:, :, :],
                        scalar1=1.0, scalar2=None, op0=ALU.bypass)

                for kb in range(n_tt + 1):
                    if kb < n_tt:
                        lo_qb = max(kb - 1, 0)
                        hi_qb = min(kb + 1, n_tt - 1)
                        ncols = (hi_qb - lo_qb + 1) * 128
                        boff2 = (lo_qb - (kb - 1)) * 128
                        for h in range(H):
                            pss = sps.tile([128, 384], FP32,
                                           name=f"pss{kb}_{h}{sfx}", tag="sps")
                            nc.tensor.matmul(
                                pss[:, :ncols], ident[:],
                                btiles[:, h * 384 + boff2:h * 384 + boff2 + ncols],
                                start=True, stop=False)
                            hrow = (h % 2) * 64
                            nc.tensor.matmul(
                                pss[:, :ncols],
                                kT[h // 2][hrow:hrow + 64, kb * 128:(kb + 1) * 128],
                                qT[h // 2][hrow:hrow + 64,
                                           lo_qb * 128:lo_qb * 128 + ncols],
                                start=False, stop=True)
                            ew = xh_p.tile([128, 384], FP8, name=f"ew{kb}_{h}{sfx}",
                                           tag=f"ew{h}", bufs=3)
                            nc.scalar.activation(ew[:, :ncols], pss[:, :ncols],
                                                 AF.Exp, scale=QS)
                            expw[(kb, h)] = (ew, lo_qb)
                    qb = kb - 1
                    if 0 <= qb < n_tt:
                        do_av(qb)

                for t in range(n_tt):
                    ps2 = second_gemm_ps(at8, t * 128, T1, 2, wo, f"o_{t}")
                    nc.vector.tensor_tensor(out=x3[:, t, :], in0=ps2[:],
                                            in1=x3[:, t, :], op=ALU.add)

            def conv_stage():
                ln_xhat("conv")
                Tc = T1
                hg = attn_p.tile([128, 4 * (Tc + 2 * PAD)], FP8, name=f"hg{sfx}", tag="abig1")
                hg3 = hg[:].rearrange("p (c n) -> p c n", c=4)
                sl8 = attn_p.tile([128, 4 * Tc], FP8, name=f"sl8{sfx}", tag="abig2")
                sl3 = sl8[:].rearrange("p (c n) -> p c n", c=4)
                for c in range(4):
                    nc.vector.memset(hg3[:, c, 0:PAD], 0.0)
                    nc.vector.memset(hg3[:, c, PAD + Tc:], 0.0)
                for (n_off, n_w) in nsplit(Tc, 1024):
                    g8 = hp.tile([128, 4 * 1024], FP8, name=f"g8_{n_off}{sfx}", tag="g8")
                    g83 = g8[:].rearrange("p (m n) -> p m n", m=4)
                    for m in range(4):
                        ps = big_ps.tile([128, 1024], FP32,
                                         name=f"psg{m}_{n_off}{sfx}", tag="big_ps")
                        first_gemm(wpw1, 0, 4 + m, n_off, n_w, ps)
                        nc.scalar.activation(g83[:, m, 0:n_w], ps[:, 0:n_w], AF.Sigmoid,
                                             bias=bcols[:, BC["pw1"] + 4 + m:
                                                        BC["pw1"] + 5 + m],
                                             scale=1.0 / WS)
                    for m in range(4):
                        ps = big_ps.tile([128, 1024], FP32,
                                         name=f"psa{m}_{n_off}{sfx}", tag="big_ps")
                        first_gemm(wpw1, 0, m, n_off, n_w, ps)
                        # hg = (32a) * sig(gate)  [fp8, 32x scale]
                        nc.vector.scalar_tensor_tensor(
                            out=hg3[:, m, PAD + n_off:PAD + n_off + n_w],
                            in0=ps[:, 0:n_w], scalar=1.0, in1=g83[:, m, 0:n_w],
                            op0=ALU.bypass, op1=ALU.mult)
                for (n_off, n_w) in nsplit(Tc, 1024):
                    for c in range(4):
                        psd = big_ps.tile([128, 1024], FP32,
                                          name=f"psd{c}_{n_off}{sfx}", tag="big_ps")
                        for j in range(16):
                            lhs = drp_n(wdw, (c * 16 + j) * 256, 128, 128)
                            for (so, sw) in nsplit(n_w, 512):
                                # taps (2j, 2j+1): hg col offset n_off+so+2j+1-PAD+PAD
                                rhs = drp_n(hg, c * (Tc + 2 * PAD) + n_off + so + 2 * j + 1,
                                            1, sw)
                                nc.tensor.matmul(psd[:, so:so + sw], lhs, rhs,
                                                 start=(j == 0), stop=(j == 15),
                                                 perf_mode=PM.DoubleRow,
                                                 skip_group_check=True)
                        nc.scalar.activation(sl3[:, c, n_off:n_off + n_w],
                                             psd[:, 0:n_w], AF.Silu,
                                             bias=bcols[:, BC["conv"] + c:BC["conv"] + c + 1],
                                             scale=1.0 / (WS * WS))
                for t in range(n_tt):
                    ps2 = second_gemm_ps(sl8, t * 128, Tc, 2, wpw2, f"pw2_{t}")
                    nc.gpsimd.tensor_tensor(out=x3[:, t, :], in0=ps2[:],
                                            in1=x3[:, t, :], op=ALU.add)

            ffn_stage(0, "ff1")
            attn_stage()
            conv_stage()
            ffn_stage(1, "ff2")
            ln_xhat("fin", fin=True)
            ysrc = y_ext[:].rearrange("(t p) d -> p t d", p=128)
            for blk in range(2):
                t0, t1_ = blk * 9, blk * 9 + 9
                nc.sync.dma_start(ysrc[:, t0:t1_, :], x3[:, t0:t1_, :])

        for rep in range(reps):
            body(rep)

    nc.compile()
    return nc


# ===================== host-side preprocessing =====================

F8 = ml_dtypes.float8_e4m3


def _pack_first(w):
    """Stationary DoubleRow layout for chan-major GEMMs.
    w: [512, dout] fp32 -> [128, (dout/128)*2*2*128]."""
    din, dout = w.shape
    nm = dout // 128
    out = np.zeros((128, nm * 512), np.float32)
    for m in range(nm):
        for p in range(2):
            for i in range(2):
                out[:, ((m * 2 + p) * 2 + i) * 128:((m * 2 + p) * 2 + i + 1) * 128] = \
                    w[(2 * p + i) * 128:(2 * p + i + 1) * 128, m * 128:(m + 1) * 128]
    return out


def _pack_moving(w):
    """Moving DoubleRow layout for token-major GEMMs.
    w: [K, 512] fp32 -> [128, (K/256)*2*512]."""
    K, dout = w.shape
    out = np.zeros((128, (K // 128) * dout), np.float32)
    for j in range(K // 256):
        for i in range(2):
            out[:, (j * 2 + i) * dout:(j * 2 + i + 1) * dout] = \
                w[(2 * j + i) * 128:(2 * j + i + 1) * 128, :]
    return out


def prep_weights(inp):
    f = lambda a: np.asarray(a, dtype=np.float32)
    out = {}
    bcols = np.zeros((128, 64), np.float32)
    brow = np.zeros((1, 2048), np.float32)

    def fold_ln(g, b, w, bias):
        return f(g)[:, None] * f(w), f(b) @ f(w) + f(bias)

    for i, p in ((0, "ff1"), (1, "ff2")):
        w1g, b1 = fold_ln(inp[p + "_ln_g"], inp[p + "_ln_b"], inp[p + "_w1"], inp[p + "_b1"])
        out[f"w_ff{i+1}a"] = _pack_first(w1g * WS).astype(F8)
        bcols[:, BC[p]:BC[p] + 16] = b1.reshape(16, 128).T
        out[f"w_ff{i+1}b"] = _pack_moving(f(inp[p + "_w2"]) * 0.5 * WS).astype(F8)
        brow[0, BR[p]:BR[p] + 512] = f(inp[p + "_b2"]) * 0.5 * WS

    g, b = inp["attn_ln_g"], inp["attn_ln_b"]
    packs = []
    for nm in ("q", "k", "v"):
        wg, bb = fold_ln(g, b, inp["w" + nm], inp["b" + nm])
        if nm == "v":
            packs.append(_pack_moving(wg * WS))
        else:
            packs.append(_pack_first(wg * WS))
            bcols[:, BC[nm]:BC[nm] + 4] = bb.reshape(4, 128).T
    out["w_qkv"] = np.concatenate(packs, axis=1).astype(F8)
    # v bias enters at 32x inside va (folded into scatter? -> via brow trick no;
    # v bias is zero in practice; fold exactly: add 32*bv to each v column via
    # the ones slot is not possible; instead add bv into wv fold: v = x_hat@wv + bv
    # -> append bias as extra contraction row is unavailable; we add it to the
    # scatter op below through va memset? Keep exact by adding bv*WS to va via
    # gpsimd scalar2 per-partition: partition dim of va is TOKENS, bias is per
    # channel -> not per-partition. bv==0 for this model family; asserted host-side.
    assert np.abs(f(inp["bv"])).max() == 0.0, "nonzero bv unsupported"
    out["w_o"] = _pack_moving(f(inp["wo"]) * WS).astype(F8)
    brow[0, BR["o"]:BR["o"] + 512] = f(inp["bo"]) * WS

    wg, bb = fold_ln(inp["conv_ln_g"], inp["conv_ln_b"], inp["pw1_w"], inp["pw1_b"])
    out["w_pw1"] = _pack_first(wg * WS).astype(F8)
    bcols[:, BC["pw1"]:BC["pw1"] + 8] = bb.reshape(8, 128).T
    out["w_pw2"] = _pack_moving(f(inp["pw2_w"]) * WS).astype(F8)
    brow[0, BR["pw2"]:BR["pw2"] + 512] = f(inp["pw2_b"]) * WS

    bn_scale = f(inp["bn_g"]) / np.sqrt(f(inp["bn_v"]) + EPS)
    dww = f(inp["dw_w"])[:, 0, :] * bn_scale[:, None] * WS   # [512, 31]
    bconv_full = (f(inp["dw_b"]) - f(inp["bn_m"])) * bn_scale + f(inp["bn_b"])
    bcols[:, BC["conv"]:BC["conv"] + 4] = bconv_full.reshape(4, 128).T
    wdw = np.zeros((128, 16384), np.float32)
    for c in range(4):
        for j in range(16):
            for i in range(2):
                k = 2 * j + i
                if k < KS:
                    blk = wdw[:, ((c * 16 + j) * 2 + i) * 128:
                              ((c * 16 + j) * 2 + i + 1) * 128]
                    np.fill_diagonal(blk, dww[c * 128:(c + 1) * 128, k])
    out["w_dw"] = wdw.astype(F8)

    rb = f(inp["rel_bias"])
    j = np.arange(128)[:, None]
    i = np.arange(128)[None, :]
    bt = np.zeros((128, H * 3 * 128), np.float32)
    for h in range(H):
        for di, delta in enumerate((1, 0, -1)):
            rel = delta * 128 + j - i
            valid = np.abs(rel) <= CTX
            idx = np.clip(rel + CTX, 0, 2 * CTX)
            bt[:, h * 384 + di * 128:h * 384 + (di + 1) * 128] = \
                np.where(valid, 8.0 * rb[h, idx], -1e30)
    out["btiles"] = bt.astype(ml_dtypes.bfloat16)
    out["ident"] = np.eye(128, dtype=np.float32).astype(ml_dtypes.bfloat16)
    out["biasrow"] = brow.astype(ml_dtypes.bfloat16)
    out["bcols"] = bcols.astype(np.float32)
    return out


# ===================== SPMD runner =====================

def _make_runner(nc, n_cores):
    import jax
    from jax.sharding import Mesh, PartitionSpec
    from jax.experimental.shard_map import shard_map
    from concourse import bass2jax
    from concourse.bass2jax import _bass_exec_p, install_neuronx_cc_hook

    install_neuronx_cc_hook()
    partition_name = nc.partition_id_tensor.name if nc.partition_id_tensor else None
    in_names, out_names, out_avals, zero_shapes = [], [], [], []
    for alloc in nc.m.functions[0].allocations:
        if not isinstance(alloc, mybir.MemoryLocationSet):
            continue
        name = alloc.memorylocations[0].name
        if alloc.kind == "ExternalInput":
            if name != partition_name:
                in_names.append(name)
        elif alloc.kind == "ExternalOutput":
            out_names.append(name)
            shape = tuple(alloc.tensor_shape)
            dtype = mybir.dt.np(alloc.dtype)
            out_avals.append(jax.core.ShapedArray(shape, dtype))
            zero_shapes.append((shape, dtype))
    n_params = len(in_names)
    n_outs = len(out_avals)
    all_in_names = list(in_names) + list(out_names)
    if partition_name is not None:
        all_in_names.append(partition_name)

    def _body(*args):
        operands = list(args)
        if partition_name is not None:
            operands.append(bass2jax.partition_id_tensor())
        outs = _bass_exec_p.bind(
            *operands, out_avals=tuple(out_avals), in_names=tuple(all_in_names),
            out_names=tuple(out_names), lowering_input_output_aliases=(),
            sim_require_finite=True, sim_require_nnan=True, nc=nc)
        return tuple(outs)

    devices = jax.devices()[:n_cores]
    mesh = Mesh(np.asarray(devices), ("core",))
    sharded = jax.jit(
        shard_map(_body, mesh=mesh,
                  in_specs=(PartitionSpec("core"),) * (n_params + n_outs),
                  out_specs=(PartitionSpec("core"),) * n_outs, check_rep=False),
        donate_argnums=tuple(range(n_params, n_params + n_outs)),
        keep_unused=True)

    def run(in_maps):
        per_core = [[np.asarray(m[n]) for n in in_names] for m in in_maps]
        concat_in = [np.concatenate([per_core[c][i] for c in range(n_cores)], axis=0)
                     for i in range(n_params)]
        concat_zeros = [np.zeros((n_cores * s[0], *s[1:]), d) for (s, d) in zero_shapes]
        out_arrs = sharded(*concat_in, *concat_zeros)
        out_arrs = [np.asarray(o) for o in out_arrs]
        return [{name: out_arrs[i].reshape(n_cores, *out_avals[i].shape)[c]
                 for i, name in enumerate(out_names)}
                for c in range(n_cores)]

    return run


_CACHE = {}


def _get_compiled(reps=1):
    key = ("main", reps)
    if key not in _CACHE:
        nc = build_core_kernel(reps=reps)
        _CACHE[key] = _make_runner(nc, N_CORES)
    return _CACHE[key]


def kernel(**inputs):
    x = np.asarray(inputs["x"], dtype=np.float32)  # [B, S, D]
    wmap = prep_weights(inputs)
    in_maps = []
    for b in range(B):
        for half in range(2):
            start = 0 if half == 0 else S - T1
            m = dict(wmap)
            m["x"] = np.ascontiguousarray(x[b, start:start + T1])
            in_maps.append(m)
    run = _get_compiled()
    res = run(in_maps)
    y = np.empty((B, S, D), dtype=np.float32)
    for idx in range(N_CORES):
        b, half = divmod(idx, 2)
        out = res[idx]["y"]  # [T1, D]
        if half == 0:
            y[b, 0:S // 2] = out[0:S // 2]
        else:
            y[b, S // 2:] = out[T1 - S // 2:]
    return y


# revision 18
# speedup vs baseline: 1.9833x; 1.0816x over previous
"""Trainium2 Bass kernel for nn_ConformerBlock (B=4, S=4096, D=512).

Sharding: 8 shards = (batch 4) x (sequence halves 2), each core owns a
2304-token slice (2048 valid + 256 halo). SPMD, no collectives.

v2 design (fp8 DoubleRow):
  - All big GEMMs run fp8e4m3 with MatmulPerfMode.DoubleRow: each
    instruction contracts 2 k-tiles at 0.5 PE-cycles/out-col (4x fewer
    PE cycles than bf16).
  - fp8 denormal avoidance: every weight matrix is stored x32. The
    residual stream x_tm is kept at 32x true scale (LayerNorm is
    scale-invariant so this costs one entry-scale pass); with that,
    second-GEMM PSUM results (32 x branch) add directly onto the 32x
    residual, and first-GEMM evacuations absorb 1/32 in the ACT scale.
  - Attention: q/k evacuated at true scale (DVE, scale 1/32); scores
    get rel-bias+mask via an identity matmul of bf16 btiles; exp on
    ACT; v kept at 32x with the softmax-denominator ones column also
    32.0 so the normalization cancels the scale.
  - Depthwise conv: 31 taps as 16 DoubleRow diagonal-pair matmuls with
    overlapping-stride moving APs (rhs dim1 stride = 1 element shift).
  - Engine balance: Gelu/Exp/Sigmoid/Silu evacs on ACT (table funcs,
    batched per stage to avoid act-table reloads; LN sqrt batched per
    stage); x_hat/LN stats/residual adds/q+k evacs on DVE; fp8
    converts, v scatter and entry-scale on Pool (gpsimd); all DMAs on
    the sync engine.
"""
import sys
sys.path.insert(0, '/opt/trn_rl_repo')
from contextlib import ExitStack

import numpy as np
import ml_dtypes

import concourse.bass as bass
import concourse.tile as tile
from concourse import bacc, mybir
from concourse.bass_types import AP

AF = mybir.ActivationFunctionType
ALU = mybir.AluOpType
PM = mybir.MatmulPerfMode
FP32 = mybir.dt.float32
BF16 = mybir.dt.bfloat16
FP8 = mybir.dt.float8e4
EPS = 1e-5

B, S = 4, 4096
D, H, CTX, FFN, KS = 512, 8, 128, 2048, 31
HD = D // H
PAD = 16
N_TT = 18          # 2304 tokens per shard
N_CORES = 8
T1 = N_TT * 128
WS = 32.0          # weight/residual scale
QS = 0.125         # 1/sqrt(HD)

# bias_cols column map (true-scale per-partition biases)
BC = {"ff1": 0, "ff2": 16, "q": 32, "k": 36, "v": 40, "pw1": 44, "conv": 52}
# biasrow segment map (32x-scale free-dim bias rows, 512 wide each)
BR = {"ff1": 0, "o": 512, "pw2": 1024, "ff2": 1536}


def nsplit(width, step=512):
    out, o = [], 0
    while o < width:
        w = min(step, width - o)
        out.append((o, w))
        o += w
    return out


def build_core_kernel(n_tt=N_TT, reps=1):
    nc = bacc.Bacc("TRN2", target_bir_lowering=False, debug=False, num_devices=1)

    x_ext = nc.dram_tensor("x", [T1, D], FP32, kind="ExternalInput").ap()
    w_ffa_ext = [nc.dram_tensor(f"w_ff{i}a", [128, 8192], FP8, kind="ExternalInput").ap()
                 for i in (1, 2)]
    w_ffb_ext = [nc.dram_tensor(f"w_ff{i}b", [128, 8192], FP8, kind="ExternalInput").ap()
                 for i in (1, 2)]
    w_qkv_ext = nc.dram_tensor("w_qkv", [128, 6144], FP8, kind="ExternalInput").ap()
    w_o_ext = nc.dram_tensor("w_o", [128, 2048], FP8, kind="ExternalInput").ap()
    w_pw1_ext = nc.dram_tensor("w_pw1", [128, 4096], FP8, kind="ExternalInput").ap()
    w_pw2_ext = nc.dram_tensor("w_pw2", [128, 2048], FP8, kind="ExternalInput").ap()
    w_dw_ext = nc.dram_tensor("w_dw", [128, 16384], FP8, kind="ExternalInput").ap()
    btiles_ext = nc.dram_tensor("btiles", [128, H * 3 * 128], BF16, kind="ExternalInput").ap()
    ident_ext = nc.dram_tensor("ident", [128, 128], BF16, kind="ExternalInput").ap()
    biasrow_ext = nc.dram_tensor("biasrow", [1, 2048], BF16, kind="ExternalInput").ap()
    bcols_ext = nc.dram_tensor("bcols", [128, 64], FP32, kind="ExternalInput").ap()
    y_ext = nc.dram_tensor("y", [T1, D], FP32, kind="ExternalOutput").ap()

    def drp_n(t, off, s1, n):
        base = t[:]
        return AP(base.tensor, base.offset + off, [list(base.ap[0]), [s1, 2], [1, n]])

    with tile.TileContext(nc) as tc, ExitStack() as es:
        pool = lambda name, bufs=1, space="SBUF": es.enter_context(
            tc.tile_pool(name=name, bufs=bufs, space=space))

        const_p = pool("const")
        wp = pool("wp")            # weights (wa/wb shared between ff1/ff2)
        resid_p = pool("resid")
        stat_p = pool("stat", bufs=2)
        xt_p = pool("xt", bufs=2)  # x8T double-buffered across stages
        xh_p = pool("xh", bufs=3)
        hp = pool("hp", bufs=2)    # h8 chunks, g8 chunks, atm tiles
        attn_p = pool("attn")
        big_ps = pool("big_ps", bufs=3, space="PSUM")    # [128,512] first gemms
        out_ps = pool("out_ps", bufs=2, space="PSUM")    # [128,512] second gemms/v/av
        sps = pool("sps", bufs=2, space="PSUM")          # [128,384] scores

        ident = const_p.tile([128, 128], BF16, name="ident")
        nc.sync.dma_start(ident[:], ident_ext[:])
        ones1 = const_p.tile([1, 128], BF16, name="ones1")
        nc.vector.memset(ones1[:], 1.0)
        biasrow = const_p.tile([1, 2048], BF16, name="biasrow")
        nc.sync.dma_start(biasrow[:], biasrow_ext[:])
        bcols = const_p.tile([128, 64], FP32, name="bcols")
        nc.sync.dma_start(bcols[:], bcols_ext[:])
        eps_col = const_p.tile([128, 1], FP32, name="eps_col")
        nc.vector.memset(eps_col[:], EPS)

        # persistent weight tiles
        wa = wp.tile([128, 8192], FP8, name="wa", tag="wa")
        wb = wp.tile([128, 8192], FP8, name="wb", tag="wb")
        wqkv = wp.tile([128, 6144], FP8, name="wqkv")
        wo = wp.tile([128, 2048], FP8, name="wo")
        wpw1 = wp.tile([128, 4096], FP8, name="wpw1")
        wpw2 = wp.tile([128, 2048], FP8, name="wpw2")
        wdw = wp.tile([128, 16384], FP8, name="wdw")
        btiles = wp.tile([128, H * 3 * 128], BF16, name="btiles")

        def body(rep):
            sfx = f"r{rep}" if reps > 1 else ""
            xbig = resid_p.tile([128, n_tt * D], FP32, name=f"xbig{sfx}", tag="xbig")
            x3 = xbig[:].rearrange("p (t d) -> p t d", t=n_tt)
            xsrc = x_ext[:].rearrange("(t p) d -> p t d", p=128)

            # x load in blocks of 3 tiles (residual stays at true scale; the
            # 1/32 weight descale is folded into every residual-add stt)
            for blk in range(6):
                t0, t1_ = blk * 3, blk * 3 + 3
                nc.sync.dma_start(x3[:, t0:t1_, :], xsrc[:, t0:t1_, :])

            # weight loads (sync engine; ff1 first so compute can start)
            nc.sync.dma_start(wa[:], w_ffa_ext[0][:])
            nc.sync.dma_start(wb[:], w_ffb_ext[0][:])
            nc.sync.dma_start(wqkv[:], w_qkv_ext[:])
            nc.sync.dma_start(btiles[:], btiles_ext[:])
            nc.sync.dma_start(wo[:], w_o_ext[:])
            nc.sync.dma_start(wpw1[:], w_pw1_ext[:])
            nc.sync.dma_start(wdw[:], w_dw_ext[:])
            nc.sync.dma_start(wpw2[:], w_pw2_ext[:])

            def new_x8T(tag):
                x8T = xt_p.tile([128, 4 * T1], FP8, name=f"x8T_{tag}{sfx}", tag="x8T")
                return x8T, x8T[:].rearrange("p (c n) -> p c n", c=4)

            def ln_xhat(tag, x8T3, fin=False):
                """LN of xbig -> bf16 transpose staging -> fp8 x8T.
                Stats batched per group of 6 tiles (one sqrt/recip/nmu each);
                xh production alternates DVE/Pool. If fin: writes y in place."""
                st2g = stat_p.tile([128, n_tt * 2], FP32, name=f"st2_{tag}{sfx}", tag="st2g")
                sigg = stat_p.tile([128, 2 * n_tt], FP32, name=f"sig_{tag}{sfx}", tag="sigg")
                nmug = stat_p.tile([128, n_tt], FP32, name=f"nmu_{tag}{sfx}", tag="nmug")
                GRP = 6
                for g0 in range(0, n_tt, GRP):
                    gn = min(GRP, n_tt - g0)
                    for t in range(g0, g0 + gn):
                        st6 = stat_p.tile([128, 6], FP32, name=f"st6_{tag}{t}{sfx}",
                                          tag="st6")
                        nc.vector.bn_stats(st6[:], x3[:, t, :])
                        nc.vector.bn_aggr(st2g[:, 2 * t:2 * t + 2], st6[:])
                    varv = AP(st2g[:].tensor, st2g[:].offset + 2 * g0 + 1,
                              [list(st2g[:].ap[0]), [2, gn]])
                    nc.scalar.activation(sigg[:, g0:g0 + gn], varv, AF.Sqrt,
                                         bias=eps_col[:])
                    nc.vector.reciprocal(sigg[:, n_tt + g0:n_tt + g0 + gn],
                                         sigg[:, g0:g0 + gn])
                    meanv = AP(st2g[:].tensor, st2g[:].offset + 2 * g0,
                               [list(st2g[:].ap[0]), [2, gn]])
                    nc.vector.scalar_tensor_tensor(
                        out=nmug[:, g0:g0 + gn], in0=meanv, scalar=-1.0,
                        in1=sigg[:, n_tt + g0:n_tt + g0 + gn],
                        op0=ALU.mult, op1=ALU.mult)
                    for t in range(g0, g0 + gn):
                        if fin:
                            nc.vector.tensor_scalar(
                                out=x3[:, t, :], in0=x3[:, t, :],
                                scalar1=sigg[:, n_tt + t:n_tt + t + 1],
                                scalar2=nmug[:, t:t + 1], op0=ALU.mult, op1=ALU.add)
                            continue
                        eng = nc.vector if t % 2 == 0 else nc.gpsimd
                        xh = xh_p.tile([128, D], BF16, name=f"xh_{tag}{t}{sfx}", tag="xh")
                        eng.tensor_scalar(
                            out=xh[:], in0=x3[:, t, :],
                            scalar1=sigg[:, n_tt + t:n_tt + t + 1],
                            scalar2=nmug[:, t:t + 1], op0=ALU.mult, op1=ALU.add)
                        xtb = xh_p.tile([128, 4 * 128], BF16,
                                        name=f"xtb_{tag}{t}{sfx}", tag="xtb")
                        xtb3 = xtb[:].rearrange("p (c n) -> p c n", c=4)
                        nc.sync.dma_start_transpose(xtb3[:, :, :], xh[:])
                        nc.gpsimd.tensor_scalar(
                            out=x8T3[:, :, t * 128:(t + 1) * 128], in0=xtb3[:, :, :],
                            scalar1=1.0, scalar2=None, op0=ALU.bypass)

            def first_gemm(x8T, wtile, wcoloff, m, n_off, n_w, ps):
                """chan-major GEMM: ps[:, :n_w] = W_m.T @ x8T[:, n_off:+n_w].
                K=512 via 2 DoubleRow pairs (n_w <= 512)."""
                for p in range(2):
                    lhs = drp_n(wtile, wcoloff + (m * 2 + p) * 256, 128, 128)
                    rhs = drp_n(x8T, (2 * p) * T1 + n_off, T1, n_w)
                    nc.tensor.matmul(ps[:, 0:n_w], lhs, rhs,
                                     start=(p == 0), stop=(p == 1),
                                     perf_mode=PM.DoubleRow)

            def second_gemm_ps(lhstile, lhs_off, lhs_s1, npair, wtile, nm):
                """token-major GEMM: ps2[tok,512] = bias + sum DR pairs."""
                ps2 = out_ps.tile([128, 512], FP32, name=f"ps2_{nm}{sfx}", tag="out_ps")
                nc.tensor.matmul(ps2[:], ones1[:], biasrow[0:1, BR[nm.split('_')[0]]:
                                 BR[nm.split('_')[0]] + 512], start=True, stop=False)
                for j in range(npair):
                    lhs = drp_n(lhstile, lhs_off + j * 2 * lhs_s1, lhs_s1, 128)
                    rhs = drp_n(wtile, j * 1024, 512, 512)
                    nc.tensor.matmul(ps2[:], lhs, rhs, start=False, stop=(j == npair - 1),
                                     perf_mode=PM.DoubleRow)
                return ps2

            def ffn_stage(idx, tag):
                if idx == 1:
                    nc.sync.dma_start(wa[:], w_ffa_ext[1][:])
                    nc.sync.dma_start(wb[:], w_ffb_ext[1][:])
                x8T, x8T3 = new_x8T(tag)
                ln_xhat(tag, x8T3)
                for (n_off, n_w) in nsplit(T1):
                    h8 = hp.tile([128, 16 * 512], FP8, name=f"h8_{tag}{n_off}{sfx}", tag="h8")
                    h83 = h8[:].rearrange("p (k n) -> p k n", k=16)
                    for m in range(16):
                        ps = big_ps.tile([128, 512], FP32,
                                         name=f"ps_{tag}{m}_{n_off}{sfx}", tag="big_ps")
                        first_gemm(x8T, wa, 0, m, n_off, n_w, ps)
                        nc.scalar.activation(h83[:, m, 0:n_w], ps[:, 0:n_w], AF.Gelu,
                                             bias=bcols[:, BC[tag] + m:BC[tag] + m + 1],
                                             scale=1.0 / WS)
                    for sub in range(n_w // 128):
                        t = (n_off + sub * 128) // 128
                        ps2 = second_gemm_ps(h8, sub * 128, 512, 8, wb, f"{tag}_{t}")
                        nc.vector.scalar_tensor_tensor(
                            out=x3[:, t, :], in0=ps2[:], scalar=1.0 / WS,
                            in1=x3[:, t, :], op0=ALU.mult, op1=ALU.add)

            def attn_stage():
                x8T, x8T3 = new_x8T("attn")
                ln_xhat("attn", x8T3)
                qkT = []   # q, k fp8 true-scale, [128, T1] x4 m-tiles each
                for pi, nm in ((0, "q"), (1, "k")):
                    tiles = []
                    for m in range(4):
                        qt = attn_p.tile([128, T1], FP8, name=f"{nm}T{m}{sfx}",
                                         tag=f"{nm}T{m}")
                        tiles.append(qt)
                        for (n_off, n_w) in nsplit(T1):
                            ps = big_ps.tile([128, 512], FP32,
                                             name=f"ps_{nm}{m}_{n_off}{sfx}", tag="big_ps")
                            first_gemm(x8T, wqkv, pi * 2048, m, n_off, n_w, ps)
                            nc.vector.tensor_scalar(
                                out=qt[:, n_off:n_off + n_w], in0=ps[:, 0:n_w],
                                scalar1=1.0 / WS,
                                scalar2=bcols[:, BC[nm] + m:BC[nm] + m + 1],
                                op0=ALU.mult, op1=ALU.add)
                    qkT.append(tiles)
                qT, kT = qkT

                # v (32x) into augmented layout, via token-major gemm per tile
                vab = attn_p.tile([128, n_tt * 8 * 65], FP8, name=f"vab{sfx}", tag="abig2")
                va3 = vab[:].rearrange("p (t h w) -> p t h w", t=n_tt, h=8)
                for t in range(n_tt):
                    ps = out_ps.tile([128, 512], FP32, name=f"psv{t}{sfx}", tag="out_ps")
                    for p in range(2):
                        lhs = drp_n(x8T, (2 * p) * T1 + t * 128, T1, 128)
                        rhs = drp_n(wqkv, 2 * 2048 + p * 1024, 512, 512)
                        nc.tensor.matmul(ps[:], lhs, rhs, start=(p == 0), stop=(p == 1),
                                         perf_mode=PM.DoubleRow)
                    # scatter 8x64 into 8x65 slots (DVE; gpsimd cannot read PSUM)
                    nc.vector.tensor_scalar(
                        out=va3[:, t, :, 0:64],
                        in0=ps[:].rearrange("p (h w) -> p h w", h=8),
                        scalar1=1.0, scalar2=None, op0=ALU.bypass)
                    nc.vector.memset(va3[:, t, :, 64:65], WS)

                at8 = attn_p.tile([128, 4 * T1], FP8, name=f"at8{sfx}", tag="abig1")
                at83 = at8[:].rearrange("p (c n) -> p c n", c=4)

                expw = {}

                def do_av(qb):
                    atm = xh_p.tile([128, D], BF16, name=f"atm{qb}{sfx}", tag="atm")
                    kbs = [kb for kb in (qb - 1, qb, qb + 1) if 0 <= kb < n_tt]
                    for hgrp in range(2):
                        pa = out_ps.tile([128, 512], FP32,
                                         name=f"pav{qb}_{hgrp}{sfx}", tag="out_ps")
                        for hh in range(4):
                            h = hgrp * 4 + hh
                            for i, kb in enumerate(kbs):
                                ew, lo_qb = expw[(kb, h)]
                                nc.tensor.matmul(
                                    pa[:, hh * 65:hh * 65 + 65],
                                    ew[:, (qb - lo_qb) * 128:(qb - lo_qb + 1) * 128],
                                    va3[:, kb, h, :],
                                    start=(i == 0), stop=(i == len(kbs) - 1),
                                    skip_group_check=True)
                        rc = stat_p.tile([128, 4], FP32, name=f"rc{qb}_{hgrp}{sfx}", tag="rc")
                        dens = AP(pa[:].tensor, pa[:].offset + 64,
                                  [list(pa[:].ap[0]), [65, 4]])
                        nc.vector.reciprocal(rc[:], dens)
                        for hh in range(4):
                            h = hgrp * 4 + hh
                            if hh % 2 == 0:
                                nc.vector.tensor_scalar(
                                    out=atm[:, h * 64:(h + 1) * 64],
                                    in0=pa[:, hh * 65:hh * 65 + 64],
                                    scalar1=rc[:, hh:hh + 1], scalar2=None, op0=ALU.mult)
                            else:
                                nc.scalar.activation(
                                    atm[:, h * 64:(h + 1) * 64],
                                    pa[:, hh * 65:hh * 65 + 64], AF.Identity,
                                    scale=rc[:, hh:hh + 1])
                    atb = xh_p.tile([128, 4 * 128], BF16, name=f"atb{qb}{sfx}", tag="xtb")
                    atb3 = atb[:].rearrange("p (c n) -> p c n", c=4)
                    nc.sync.dma_start_transpose(atb3[:, :, :], atm[:])
                    nc.gpsimd.tensor_scalar(
                        out=at83[:, :, qb * 128:(qb + 1) * 128], in0=atb3[:, :, :],
                        scalar1=1.0, scalar2=None, op0=ALU.bypass)

                for kb in range(n_tt + 1):
                    if kb < n_tt:
                        lo_qb = max(kb - 1, 0)
                        hi_qb = min(kb + 1, n_tt - 1)
                        ncols = (hi_qb - lo_qb + 1) * 128
                        boff2 = (lo_qb - (kb - 1)) * 128
                        for h in range(H):
                            pss = sps.tile([128, 384], FP32,
                                           name=f"pss{kb}_{h}{sfx}", tag="sps")
                            nc.tensor.matmul(
                                pss[:, :ncols], ident[:],
                                btiles[:, h * 384 + boff2:h * 384 + boff2 + ncols],
                                start=True, stop=False)
                            hrow = (h % 2) * 64
                            nc.tensor.matmul(
                                pss[:, :ncols],
                                kT[h // 2][hrow:hrow + 64, kb * 128:(kb + 1) * 128],
                                qT[h // 2][hrow:hrow + 64,
                                           lo_qb * 128:lo_qb * 128 + ncols],
                                start=False, stop=True)
                            ew = xh_p.tile([128, 384], FP8, name=f"ew{kb}_{h}{sfx}",
                                           tag=f"ew{h}", bufs=3)
                            nc.scalar.activation(ew[:, :ncols], pss[:, :ncols],
                                                 AF.Exp, scale=QS)
                            expw[(kb, h)] = (ew, lo_qb)
                    qb = kb - 1
                    if 0 <= qb < n_tt:
                        do_av(qb)

                for t in range(n_tt):
                    ps2 = second_gemm_ps(at8, t * 128, T1, 2, wo, f"o_{t}")
                    nc.vector.scalar_tensor_tensor(
                        out=x3[:, t, :], in0=ps2[:], scalar=1.0 / WS,
                        in1=x3[:, t, :], op0=ALU.mult, op1=ALU.add)

            def conv_stage():
                x8T, x8T3 = new_x8T("conv")
                ln_xhat("conv", x8T3)
                Tc = T1
                hg = attn_p.tile([128, 4 * (Tc + 2 * PAD)], FP8, name=f"hg{sfx}", tag="abig1")
                hg3 = hg[:].rearrange("p (c n) -> p c n", c=4)
                sl8 = attn_p.tile([128, 4 * Tc], FP8, name=f"sl8{sfx}", tag="abig2")
                sl3 = sl8[:].rearrange("p (c n) -> p c n", c=4)
                for c in range(4):
                    nc.vector.memset(hg3[:, c, 0:PAD], 0.0)
                    nc.vector.memset(hg3[:, c, PAD + Tc:], 0.0)
                for (n_off, n_w) in nsplit(Tc):
                    g8 = hp.tile([128, 4 * 512], FP8, name=f"g8_{n_off}{sfx}", tag="g8")
                    g83 = g8[:].rearrange("p (m n) -> p m n", m=4)
                    for m in range(4):
                        ps = big_ps.tile([128, 512], FP32,
                                         name=f"psg{m}_{n_off}{sfx}", tag="big_ps")
                        first_gemm(x8T, wpw1, 0, 4 + m, n_off, n_w, ps)
                        nc.scalar.activation(g83[:, m, 0:n_w], ps[:, 0:n_w], AF.Sigmoid,
                                             bias=bcols[:, BC["pw1"] + 4 + m:
                                                        BC["pw1"] + 5 + m],
                                             scale=1.0 / WS)
                    for m in range(4):
                        ps = big_ps.tile([128, 512], FP32,
                                         name=f"psa{m}_{n_off}{sfx}", tag="big_ps")
                        first_gemm(x8T, wpw1, 0, m, n_off, n_w, ps)
                        # hg = (32a) * sig(gate)  [fp8, 32x scale]
                        nc.vector.scalar_tensor_tensor(
                            out=hg3[:, m, PAD + n_off:PAD + n_off + n_w],
                            in0=ps[:, 0:n_w], scalar=1.0, in1=g83[:, m, 0:n_w],
                            op0=ALU.bypass, op1=ALU.mult)
                for (n_off, n_w) in nsplit(Tc):
                    for c in range(4):
                        psd = big_ps.tile([128, 512], FP32,
                                          name=f"psd{c}_{n_off}{sfx}", tag="big_ps")
                        for j in range(16):
                            lhs = drp_n(wdw, (c * 16 + j) * 256, 128, 128)
                            rhs = drp_n(hg, c * (Tc + 2 * PAD) + n_off + 2 * j + 1,
                                        1, n_w)
                            nc.tensor.matmul(psd[:, 0:n_w], lhs, rhs,
                                             start=(j == 0), stop=(j == 15),
                                             perf_mode=PM.DoubleRow)
                        nc.scalar.activation(sl3[:, c, n_off:n_off + n_w],
                                             psd[:, 0:n_w], AF.Silu,
                                             bias=bcols[:, BC["conv"] + c:BC["conv"] + c + 1],
                                             scale=1.0 / (WS * WS))
                for t in range(n_tt):
                    ps2 = second_gemm_ps(sl8, t * 128, Tc, 2, wpw2, f"pw2_{t}")
                    nc.vector.scalar_tensor_tensor(
                        out=x3[:, t, :], in0=ps2[:], scalar=1.0 / WS,
                        in1=x3[:, t, :], op0=ALU.mult, op1=ALU.add)

            ffn_stage(0, "ff1")
            attn_stage()
            conv_stage()
            ffn_stage(1, "ff2")
            ln_xhat("fin", None, fin=True)
            ysrc = y_ext[:].rearrange("(t p) d -> p t d", p=128)
            for blk in range(2):
                t0, t1_ = blk * 9, blk * 9 + 9
                nc.sync.dma_start(ysrc[:, t0:t1_, :], x3[:, t0:t1_, :])

        for rep in range(reps):
            body(rep)

    nc.compile()
    return nc


# ===================== host-side preprocessing =====================

F8 = ml_dtypes.float8_e4m3


def _pack_first(w):
    """Stationary DoubleRow layout for chan-major GEMMs.
    w: [512, dout] fp32 -> [128, (dout/128)*2*2*128]."""
    din, dout = w.shape
    nm = dout // 128
    out = np.zeros((128, nm * 512), np.float32)
    for m in range(nm):
        for p in range(2):
            for i in range(2):
                out[:, ((m * 2 + p) * 2 + i) * 128:((m * 2 + p) * 2 + i + 1) * 128] = \
                    w[(2 * p + i) * 128:(2 * p + i + 1) * 128, m * 128:(m + 1) * 128]
    return out


def _pack_moving(w):
    """Moving DoubleRow layout for token-major GEMMs.
    w: [K, 512] fp32 -> [128, (K/256)*2*512]."""
    K, dout = w.shape
    out = np.zeros((128, (K // 128) * dout), np.float32)
    for j in range(K // 256):
        for i in range(2):
            out[:, (j * 2 + i) * dout:(j * 2 + i + 1) * dout] = \
                w[(2 * j + i) * 128:(2 * j + i + 1) * 128, :]
    return out


def prep_weights(inp):
    f = lambda a: np.asarray(a, dtype=np.float32)
    out = {}
    bcols = np.zeros((128, 64), np.float32)
    brow = np.zeros((1, 2048), np.float32)

    def fold_ln(g, b, w, bias):
        return f(g)[:, None] * f(w), f(b) @ f(w) + f(bias)

    for i, p in ((0, "ff1"), (1, "ff2")):
        w1g, b1 = fold_ln(inp[p + "_ln_g"], inp[p + "_ln_b"], inp[p + "_w1"], inp[p + "_b1"])
        out[f"w_ff{i+1}a"] = _pack_first(w1g * WS).astype(F8)
        bcols[:, BC[p]:BC[p] + 16] = b1.reshape(16, 128).T
        out[f"w_ff{i+1}b"] = _pack_moving(f(inp[p + "_w2"]) * 0.5 * WS).astype(F8)
        brow[0, BR[p]:BR[p] + 512] = f(inp[p + "_b2"]) * 0.5 * WS

    g, b = inp["attn_ln_g"], inp["attn_ln_b"]
    packs = []
    for nm in ("q", "k", "v"):
        wg, bb = fold_ln(g, b, inp["w" + nm], inp["b" + nm])
        if nm == "v":
            packs.append(_pack_moving(wg * WS))
        else:
            packs.append(_pack_first(wg * WS))
            bcols[:, BC[nm]:BC[nm] + 4] = bb.reshape(4, 128).T
    out["w_qkv"] = np.concatenate(packs, axis=1).astype(F8)
    # v bias enters at 32x inside va (folded into scatter? -> via brow trick no;
    # v bias is zero in practice; fold exactly: add 32*bv to each v column via
    # the ones slot is not possible; instead add bv into wv fold: v = x_hat@wv + bv
    # -> append bias as extra contraction row is unavailable; we add it to the
    # scatter op below through va memset? Keep exact by adding bv*WS to va via
    # gpsimd scalar2 per-partition: partition dim of va is TOKENS, bias is per
    # channel -> not per-partition. bv==0 for this model family; asserted host-side.
    assert np.abs(f(inp["bv"])).max() == 0.0, "nonzero bv unsupported"
    out["w_o"] = _pack_moving(f(inp["wo"]) * WS).astype(F8)
    brow[0, BR["o"]:BR["o"] + 512] = f(inp["bo"]) * WS

    wg, bb = fold_ln(inp["conv_ln_g"], inp["conv_ln_b"], inp["pw1_w"], inp["pw1_b"])
    out["w_pw1"] = _pack_first(wg * WS).astype(F8)
    bcols[:, BC["pw1"]:BC["pw1"] + 8] = bb.reshape(8, 128).T
    out["w_pw2"] = _pack_moving(f(inp["pw2_w"]) * WS).astype(F8)
    brow[0, BR["pw2"]:BR["pw2"] + 512] = f(inp["pw2_b"]) * WS

    bn_scale = f(inp["bn_g"]) / np.sqrt(f(inp["bn_v"]) + EPS)
    dww = f(inp["dw_w"])[:, 0, :] * bn_scale[:, None] * WS   # [512, 31]
    bconv_full = (f(inp["dw_b"]) - f(inp["bn_m"])) * bn_scale + f(inp["bn_b"])
    bcols[:, BC["conv"]:BC["conv"] + 4] = bconv_full.reshape(4, 128).T
    wdw = np.zeros((128, 16384), np.float32)
    for c in range(4):
        for j in range(16):
            for i in range(2):
                k = 2 * j + i
                if k < KS:
                    blk = wdw[:, ((c * 16 + j) * 2 + i) * 128:
                              ((c * 16 + j) * 2 + i + 1) * 128]
                    np.fill_diagonal(blk, dww[c * 128:(c + 1) * 128, k])
    out["w_dw"] = wdw.astype(F8)

    rb = f(inp["rel_bias"])
    j = np.arange(128)[:, None]
    i = np.arange(128)[None, :]
    bt = np.zeros((128, H * 3 * 128), np.float32)
    for h in range(H):
        for di, delta in enumerate((1, 0, -1)):
            rel = delta * 128 + j - i
            valid = np.abs(rel) <= CTX
            idx = np.clip(rel + CTX, 0, 2 * CTX)
            bt[:, h * 384 + di * 128:h * 384 + (di + 1) * 128] = \
                np.where(valid, 8.0 * rb[h, idx], -1e30)
    out["btiles"] = bt.astype(ml_dtypes.bfloat16)
    out["ident"] = np.eye(128, dtype=np.float32).astype(ml_dtypes.bfloat16)
    out["biasrow"] = brow.astype(ml_dtypes.bfloat16)
    out["bcols"] = bcols.astype(np.float32)
    return out


# ===================== SPMD runner =====================

def _make_runner(nc, n_cores):
    import jax
    from jax.sharding import Mesh, PartitionSpec
    from jax.experimental.shard_map import shard_map
    from concourse import bass2jax
    from concourse.bass2jax import _bass_exec_p, install_neuronx_cc_hook

    install_neuronx_cc_hook()
    partition_name = nc.partition_id_tensor.name if nc.partition_id_tensor else None
    in_names, out_names, out_avals, zero_shapes = [], [], [], []
    for alloc in nc.m.functions[0].allocations:
        if not isinstance(alloc, mybir.MemoryLocationSet):
            continue
        name = alloc.memorylocations[0].name
        if alloc.kind == "ExternalInput":
            if name != partition_name:
                in_names.append(name)
        elif alloc.kind == "ExternalOutput":
            out_names.append(name)
            shape = tuple(alloc.tensor_shape)
            dtype = mybir.dt.np(alloc.dtype)
            out_avals.append(jax.core.ShapedArray(shape, dtype))
            zero_shapes.append((shape, dtype))
    n_params = len(in_names)
    n_outs = len(out_avals)
    all_in_names = list(in_names) + list(out_names)
    if partition_name is not None:
        all_in_names.append(partition_name)

    def _body(*args):
        operands = list(args)
        if partition_name is not None:
            operands.append(bass2jax.partition_id_tensor())
        outs = _bass_exec_p.bind(
            *operands, out_avals=tuple(out_avals), in_names=tuple(all_in_names),
            out_names=tuple(out_names), lowering_input_output_aliases=(),
            sim_require_finite=True, sim_require_nnan=True, nc=nc)
        return tuple(outs)

    devices = jax.devices()[:n_cores]
    mesh = Mesh(np.asarray(devices), ("core",))
    sharded = jax.jit(
        shard_map(_body, mesh=mesh,
                  in_specs=(PartitionSpec("core"),) * (n_params + n_outs),
                  out_specs=(PartitionSpec("core"),) * n_outs, check_rep=False),
        donate_argnums=tuple(range(n_params, n_params + n_outs)),
        keep_unused=True)

    def run(in_maps):
        per_core = [[np.asarray(m[n]) for n in in_names] for m in in_maps]
        concat_in = [np.concatenate([per_core[c][i] for c in range(n_cores)], axis=0)
                     for i in range(n_params)]
        concat_zeros = [np.zeros((n_cores * s[0], *s[1:]), d) for (s, d) in zero_shapes]
        out_arrs = sharded(*concat_in, *concat_zeros)
        out_arrs = [np.asarray(o) for o in out_arrs]
        return [{name: out_arrs[i].reshape(n_cores, *out_avals[i].shape)[c]
                 for i, name in enumerate(out_names)}
                for c in range(n_cores)]

    return run


_CACHE = {}


def _get_compiled(reps=1):
    key = ("main", reps)
    if key not in _CACHE:
        nc = build_core_kernel(reps=reps)
        _CACHE[key] = _make_runner(nc, N_CORES)
    return _CACHE[key]


def kernel(**inputs):
    x = np.asarray(inputs["x"], dtype=np.float32)  # [B, S, D]
    wmap = prep_weights(inputs)
    in_maps = []
    for b in range(B):
        for half in range(2):
            start = 0 if half == 0 else S - T1
            m = dict(wmap)
            m["x"] = np.ascontiguousarray(x[b, start:start + T1])
            in_maps.append(m)
    run = _get_compiled()
    res = run(in_maps)
    y = np.empty((B, S, D), dtype=np.float32)
    for idx in range(N_CORES):
        b, half = divmod(idx, 2)
        out = res[idx]["y"]  # [T1, D]
        if half == 0:
            y[b, 0:S // 2] = out[0:S // 2]
        else:
            y[b, S // 2:] = out[T1 - S // 2:]
    return y
